# revision 1
# baseline (speedup 1.0000x reference)
"""Distributed GAT (2x GATConv + MLP self-path) on 8 Trainium2 NeuronCores.

Strategy (dst-node graph parallelism, SPMD on 8 cores):
  Host:
    - fold attention vectors into projection weights (a_s = x @ v_src etc),
      transpose x, append self-loops, sort edges by dst, partition edges by
      dst-node block of N/8 per core, group each core's dst nodes into
      128-node groups, order each group's edges [src<LO | src>=LO] with both
      sections padded to 128-edge chunks (section sizes are global constants
      so the SPMD program is identical across cores).
  Device:
    - Phase S: MLP self path on own node block.
    - Phase T: merged projection table  row(n) = [xp1|a_s1|xp2|a_s2] (1088
      f32) written as two tables (nodes < LO and >= LO, so dma_gather's
      int16 indices reach every row), plus paired a_d table
      adP[r] = [a_d1(2r)|a_d2(2r)|a_d1(2r+1)|a_d2(2r+1)] indexed by dst//2.
    - Phase E: per 128-dst-node group: dma_gather table rows for all edges
      (one pass for both convs), scores exp(leakyrelu(a_s+a_d)) without
      max-subtraction (scores are O(5), exact softmax identity), one-hot
      segment matmuls accumulate both the weighted feature sum and the
      softmax denominator in PSUM; normalisation, head-mean, bias, elu.
  Host: concatenate per-core dst blocks.
"""
import math
import numpy as np

import concourse.bass as bass
import concourse.tile as tile
import concourse.mybir as mybir
import bass_rust
from concourse import bacc
from concourse.bass_utils import run_bass_kernel_spmd

AX_X = bass_rust.AxisListType.X
F32 = mybir.dt.float32
F32R = mybir.dt.float32r
I16 = mybir.dt.int16
Act = mybir.ActivationFunctionType
Alu = mybir.AluOpType
P = 128


class Cfg:
    def __init__(self, n=50000, d_in=256, out=16, heads=32, n_cores=8, sub=5,
                 lo=32768):
        self.n = n
        self.d_in = d_in
        self.c = out
        self.h = heads
        self.hd = heads * out          # 512
        self.tw = self.hd + heads      # per-conv table cols: xp|a_s (544)
        self.row = 2 * self.tw         # merged gather row (1088 f32)
        self.tcols = self.tw + heads   # matmul out cols per conv (576)
        self.n_cores = n_cores
        self.npc = n // n_cores
        self.groups = math.ceil(self.npc / P)
        self.npc_pad = self.groups * P
        self.nt = math.ceil(n / P)
        self.n_pad = self.nt * P
        self.kd = d_in // P
        self.sub = sub
        self.lo = lo                   # node-id split for int16 gather tables
        assert lo % P == 0 and lo < self.n_pad
        self.ntl = lo // P             # table tiles in lo table
        self.n_hi = self.n_pad - lo


def _elu(nc, pool, out_ap, in_ap, tag):
    shape = list(in_ap.shape)
    u = pool.tile(shape, F32, tag=tag + "_u")
    rl = pool.tile(shape, F32, tag=tag + "_r")
    nc.vector.tensor_scalar_min(out=u[:], in0=in_ap, scalar1=0.0)
    nc.scalar.activation(u[:], u[:], Act.Exp)
    nc.scalar.activation(rl[:], in_ap, Act.Relu)
    nc.vector.scalar_tensor_tensor(
        out=out_ap, in0=u[:], scalar=-1.0, in1=rl[:], op0=Alu.add, op1=Alu.add)


def build_program(cfg: Cfg, ch_lo: int, ch_hi: int):
    nc = bacc.Bacc("TRN2", target_bir_lowering=False, debug=False,
                   num_devices=cfg.n_cores)
    g, H, C, HD, TW, TC = cfg.groups, cfg.h, cfg.c, cfg.hd, cfg.tw, cfg.tcols
    ROW = cfg.row
    ch = ch_lo + ch_hi

    t_xT = nc.dram_tensor("xT", [cfg.d_in, cfg.n_pad], F32, kind="ExternalInput")
    t_xTs = nc.dram_tensor("xTs", [cfg.d_in, cfg.npc_pad], F32, kind="ExternalInput")
    t_wc = [nc.dram_tensor(f"wc{i}", [cfg.d_in, TC], F32, kind="ExternalInput")
            for i in range(2)]
    t_l1w = nc.dram_tensor("l1w", [cfg.d_in, 4 * C], F32, kind="ExternalInput")
    t_l2w = nc.dram_tensor("l2w", [4 * C, C], F32, kind="ExternalInput")
    t_l1b = nc.dram_tensor("l1b", [4 * C, 1], F32, kind="ExternalInput")
    t_l2b = nc.dram_tensor("l2b", [P, C], F32, kind="ExternalInput")
    t_b = [nc.dram_tensor(f"b{i}", [P, C], F32, kind="ExternalInput")
           for i in range(2)]
    t_esrc = nc.dram_tensor("esrc", [P, g * ch * 8], I16, kind="ExternalInput")
    t_edst = nc.dram_tensor("edst", [P, g * ch * 8], I16, kind="ExternalInput")
    t_erel = nc.dram_tensor("erel", [P, g * ch], F32, kind="ExternalInput")
    t_epar = nc.dram_tensor("epar", [P, g * ch], F32, kind="ExternalInput")

    t_oconv = [nc.dram_tensor(f"out_conv{i}", [cfg.npc_pad, C], F32,
                              kind="ExternalOutput") for i in range(2)]
    t_oself = nc.dram_tensor("out_self", [cfg.npc_pad, C], F32,
                             kind="ExternalOutput")

    t_tabL = nc.dram_tensor("tabL", [cfg.lo, ROW], F32)
    t_tabH = nc.dram_tensor("tabH", [cfg.n_hi, ROW], F32)
    t_adP = nc.dram_tensor("adP", [cfg.n_pad // 2, 4 * H], F32)

    with tile.TileContext(nc) as tc:
        import contextlib
        with contextlib.ExitStack() as ctx:
            cst = ctx.enter_context(tc.tile_pool(name="cst", bufs=1))
            ldp = ctx.enter_context(tc.tile_pool(name="ldp", bufs=2))
            xtp = ctx.enter_context(tc.tile_pool(name="xtp", bufs=2))
            tbp = ctx.enter_context(tc.tile_pool(name="tbp", bufs=3))
            gat = ctx.enter_context(tc.tile_pool(name="gat", bufs=2))
            wkp = ctx.enter_context(tc.tile_pool(name="wkp", bufs=2))
            psB = ctx.enter_context(tc.tile_pool(name="psB", bufs=2, space="PSUM"))
            psC = ctx.enter_context(tc.tile_pool(name="psC", bufs=1, space="PSUM"))

            # ---- constants ----
            iota_i = cst.tile([P, P], mybir.dt.int32, tag="ioi")
            nc.gpsimd.iota(iota_i[:], pattern=[[1, P]], base=0, channel_multiplier=0)
            iota_f = cst.tile([P, P], F32, tag="iof")
            nc.vector.tensor_copy(iota_f[:], iota_i[:])
            wt = []
            for i in range(2):
                wk = []
                for k in range(cfg.kd):
                    w0 = ldp.tile([P, TC], F32, tag="wld")
                    nc.sync.dma_start(out=w0[:], in_=t_wc[i][k * P:(k + 1) * P, :])
                    w = cst.tile([P, TC], F32R, tag=f"w{i}{k}")
                    nc.vector.tensor_copy(w[:], w0[:])
                    wk.append(w)
                wt.append(wk)
            l1w = []
            for k in range(cfg.kd):
                w0 = ldp.tile([P, 4 * C], F32, tag="wld2")
                nc.sync.dma_start(out=w0[:], in_=t_l1w[k * P:(k + 1) * P, :])
                w = cst.tile([P, 4 * C], F32R, tag=f"l1w{k}")
                nc.vector.tensor_copy(w[:], w0[:])
                l1w.append(w)
            l2w0 = ldp.tile([4 * C, C], F32, tag="wld3")
            nc.sync.dma_start(out=l2w0[:], in_=t_l2w[:])
            l2w = cst.tile([4 * C, C], F32R, tag="l2w")
            nc.vector.tensor_copy(l2w[:], l2w0[:])
            l1b = cst.tile([4 * C, 1], F32, tag="l1b")
            nc.sync.dma_start(out=l1b[:], in_=t_l1b[:])
            l2b = cst.tile([P, C], F32, tag="l2b")
            nc.sync.dma_start(out=l2b[:], in_=t_l2b[:])
            bt = []
            for i in range(2):
                b = cst.tile([P, C], F32, tag=f"bt{i}")
                nc.sync.dma_start(out=b[:], in_=t_b[i][:])
                bt.append(b)
            erel = cst.tile([P, g * ch], F32, tag="erel")
            nc.sync.dma_start(out=erel[:], in_=t_erel[:])
            epar = cst.tile([P, g * ch], F32, tag="epar")
            nc.sync.dma_start(out=epar[:], in_=t_epar[:])

            # ---- Phase S: self path ----
            blk = 0
            while blk < cfg.npc_pad:
                bs = min(512, cfg.npc_pad - blk)
                x1p = psB.tile([4 * C, 512], F32, tag="outp0")
                for k in range(cfg.kd):
                    xts0 = xtp.tile([P, 512], F32, tag="xts0")
                    nc.sync.dma_start(out=xts0[:, :bs],
                                      in_=t_xTs[k * P:(k + 1) * P, blk:blk + bs])
                    xts = xtp.tile([P, 512], F32R, tag="xts")
                    nc.vector.tensor_copy(xts[:, :bs], xts0[:, :bs])
                    nc.tensor.matmul(out=x1p[:, :bs], lhsT=l1w[k][:],
                                     rhs=xts[:, :bs],
                                     start=(k == 0), stop=(k == cfg.kd - 1))
                x1s = wkp.tile([4 * C, 512], F32R, tag="x1s")
                nc.vector.tensor_add(out=x1s[:, :bs], in0=x1p[:, :bs],
                                     in1=l1b[:].to_broadcast([4 * C, bs]))
                _elu(nc, wkp, x1s[:, :bs], x1s[:, :bs], "se")
                for m in range(bs // P):
                    o2p = psC.tile([P, C], F32, tag="sump0")
                    nc.tensor.matmul(out=o2p[:],
                                     lhsT=x1s[:, m * P:(m + 1) * P],
                                     rhs=l2w[:], start=True, stop=True)
                    o2s = wkp.tile([P, C], F32, tag="o2s")
                    nc.vector.tensor_add(out=o2s[:], in0=o2p[:], in1=l2b[:])
                    _elu(nc, wkp, o2s[:], o2s[:], "so")
                    nc.sync.dma_start(
                        out=t_oself[blk + m * P:blk + (m + 1) * P, :], in_=o2s[:])
                blk += bs

            # ---- Phase T: merged tables (4 node-tiles per load) ----
            with tc.tile_pool(name="psA", bufs=1, space="PSUM") as psA:
                for ts4 in range(0, cfg.nt, 4):
                    mt = min(4, cfg.nt - ts4)
                    xk = []
                    for k in range(cfg.kd):
                        xt0 = xtp.tile([P, 512], F32, tag="xt0")
                        nc.sync.dma_start(
                            out=xt0[:, :mt * P],
                            in_=t_xT[k * P:(k + 1) * P, ts4 * P:(ts4 + mt) * P])
                        xt = xtp.tile([P, 512], F32R, tag=f"xt{k}")
                        nc.vector.tensor_copy(xt[:, :mt * P], xt0[:, :mt * P])
                        xk.append(xt)
                    for m in range(mt):
                        ts = ts4 + m
                        for i in range(2):
                            hw = TC // 2  # 288
                            pt = psA.tile([P, 1024], F32, tag="tps")
                            for k in range(cfg.kd):
                                lhs = xk[k][:, m * P:(m + 1) * P]
                                nc.tensor.matmul(out=pt[:, 0:hw], lhsT=lhs,
                                                 rhs=wt[i][k][:, 0:hw],
                                                 start=(k == 0),
                                                 stop=(k == cfg.kd - 1))
                                nc.tensor.matmul(out=pt[:, 512:512 + hw], lhsT=lhs,
                                                 rhs=wt[i][k][:, hw:TC],
                                                 start=(k == 0),
                                                 stop=(k == cfg.kd - 1))
                            tb = tbp.tile([P, TC], F32, tag="tb")
                            nc.vector.tensor_copy(
                                out=tb[:].rearrange("p (b x) -> p b x", b=2),
                                in_=pt[:].rearrange("p (b x) -> p b x", b=2)[:, :, 0:hw])
                            if ts < cfg.ntl:
                                tab_rows = t_tabL[ts * P:(ts + 1) * P, :]
                            else:
                                tab_rows = t_tabH[(ts - cfg.ntl) * P:
                                                  (ts - cfg.ntl + 1) * P, :]
                            nc.sync.dma_start(out=tab_rows[:, i * TW:(i + 1) * TW],
                                              in_=tb[:, 0:TW])
                            # a_d cols -> paired adP[t*64+q//2, (q%2)*64+i*32 :]
                            adv = t_adP[ts * (P // 2):(ts + 1) * (P // 2), :] \
                                .rearrange("r (o c2 e) -> r o c2 e", o=2, c2=2)
                            nc.sync.dma_start(out=adv[:, :, i, :], in_=tb[:, TW:TC])

            # ---- Phase E: edge aggregation (both convs per group) ----
            SUB = cfg.sub
            # per-section sub-ranges (static)
            spans = []
            for s0, s1, lo_flag in [(0, ch_lo, True), (ch_lo, ch, False)]:
                c0 = s0
                while c0 < s1:
                    jw = min(SUB, s1 - c0)
                    spans.append((c0, jw, lo_flag))
                    c0 += jw
            for gi in range(g):
                isrc = gat.tile([P, ch * 8], I16, tag="isrc")
                nc.sync.dma_start(out=isrc[:],
                                  in_=t_esrc[:, gi * ch * 8:(gi + 1) * ch * 8])
                idst = gat.tile([P, ch * 8], I16, tag="idst")
                nc.sync.dma_start(out=idst[:],
                                  in_=t_edst[:, gi * ch * 8:(gi + 1) * ch * 8])
                adg2 = gat.tile([P, ch, 4 * H], F32, tag="adg2")
                for (c0, jw, _lo) in spans:
                    nc.gpsimd.dma_gather(
                        out_ap=adg2[:, c0:c0 + jw, :], in_ap=t_adP[:],
                        idxs_ap=idst[:, c0 * 8:(c0 + jw) * 8],
                        num_idxs=jw * P, num_idxs_reg=jw * P, elem_size=4 * H)
                # parity select: adg = lo + par*(hi-lo)   [P, ch, 2H]
                add = wkp.tile([P, ch, 2 * H], F32, tag="add")
                nc.vector.tensor_sub(out=add[:], in0=adg2[:, :, 2 * H:],
                                     in1=adg2[:, :, 0:2 * H])
                adg = gat.tile([P, ch, 2 * H], F32, tag="adg")
                nc.vector.tensor_tensor(
                    out=adg[:], in0=add[:],
                    in1=epar[:, gi * ch:(gi + 1) * ch].unsqueeze(2)
                        .to_broadcast([P, ch, 2 * H]),
                    op=Alu.mult)
                nc.vector.tensor_add(out=adg[:], in0=adg[:],
                                     in1=adg2[:, :, 0:2 * H])

                outp = [psB.tile([P, HD], F32, tag=f"outp{i}", name=f"outp{i}")
                        for i in range(2)]
                sump = [psC.tile([P, H], F32, tag=f"sump{i}", name=f"sump{i}")
                        for i in range(2)]
                jglob = 0
                for (c0, jw, lo_flag) in spans:
                    xpa = gat.tile([P, SUB, ROW], F32, tag="xpa")
                    nc.gpsimd.dma_gather(
                        out_ap=xpa[:, :jw, :],
                        in_ap=(t_tabL[:] if lo_flag else t_tabH[:]),
                        idxs_ap=isrc[:, c0 * 8:(c0 + jw) * 8],
                        num_idxs=jw * P, num_idxs_reg=jw * P, elem_size=ROW)
                    S = wkp.tile([P, SUB, P], F32R, tag="S")
                    nc.vector.tensor_tensor(
                        out=S[:, :jw, :],
                        in0=erel[:, gi * ch + c0:gi * ch + c0 + jw]
                            .unsqueeze(2).to_broadcast([P, jw, P]),
                        in1=iota_f[:].unsqueeze(1).to_broadcast([P, jw, P]),
                        op=Alu.is_equal)
                    for i in range(2):
                        sc = wkp.tile([P, SUB, H], F32R, tag=f"sc{i}")
                        nc.vector.tensor_add(
                            out=sc[:, :jw, :],
                            in0=xpa[:, :jw, i * TW + HD:(i + 1) * TW],
                            in1=adg[:, c0:c0 + jw, i * H:(i + 1) * H])
                        nc.vector.scalar_tensor_tensor(
                            out=sc[:, :jw, :], in0=sc[:, :jw, :], scalar=0.2,
                            in1=sc[:, :jw, :], op0=Alu.mult, op1=Alu.max)
                        nc.scalar.activation(sc[:, :jw, :], sc[:, :jw, :], Act.Exp)
                        wgt = wkp.tile([P, SUB, HD], F32R, tag=f"wgt{i}")
                        nc.vector.tensor_tensor(
                            out=wgt[:, :jw, :].rearrange(
                                "p s (h c) -> p s h c", h=H),
                            in0=xpa[:, :jw, i * TW:i * TW + HD].rearrange(
                                "p s (h c) -> p s h c", h=H),
                            in1=sc[:, :jw, :].unsqueeze(3)
                                .to_broadcast([P, jw, H, C]),
                            op=Alu.mult)
                        for j in range(jw):
                            st = (jglob + j == 0)
                            sp = (jglob + j == ch - 1)
                            nc.tensor.matmul(out=outp[i][:], lhsT=S[:, j, :],
                                             rhs=wgt[:, j, :], start=st, stop=sp)
                            nc.tensor.matmul(out=sump[i][:],
                                             lhsT=S[:, j, :], rhs=sc[:, j, :],
                                             start=st, stop=sp)
                    jglob += jw
                for i in range(2):
                    rsb = wkp.tile([P, H], F32, tag=f"rsb{i}")
                    nc.scalar.activation(rsb[:], sump[i][:],
                                         Act.Copy, scale=float(H),
                                         bias=float(H) * 1e-16)
                    nc.vector.reciprocal(rsb[:], rsb[:])
                    tmp = wkp.tile([P, H, C], F32, tag=f"tmp{i}")
                    nc.vector.tensor_tensor(
                        out=tmp[:],
                        in0=outp[i][:].rearrange("p (h c) -> p h c", h=H),
                        in1=rsb[:].unsqueeze(2).to_broadcast([P, H, C]),
                        op=Alu.mult)
                    om = wkp.tile([P, C], F32, tag=f"om{i}")
                    nc.vector.reduce_sum(out=om[:],
                                         in_=tmp[:].rearrange("p h c -> p c h"),
                                         axis=AX_X)
                    nc.vector.tensor_add(out=om[:], in0=om[:], in1=bt[i][:])
                    _elu(nc, wkp, om[:], om[:], f"oe{i}")
                    nc.sync.dma_start(out=t_oconv[i][gi * P:(gi + 1) * P, :],
                                      in_=om[:])
    nc.compile()
    return nc


def _wrap16(flat):
    """edge i -> [i%16, i//16], replicated to 128 partitions."""
    w = flat.reshape(-1, 16).T  # [16, len/16]
    return np.tile(w, (8, 1))


def preprocess(cfg: Cfg, inputs):
    n, H, C = cfg.n, cfg.h, cfg.c
    x = np.asarray(inputs["x"], np.float32)
    ei = np.asarray(inputs["edge_index"])

    def fold(W, a_s, a_d):
        W = np.asarray(W, np.float32)
        v_s = (W.reshape(cfg.d_in, H, C) * np.asarray(a_s, np.float32)[None]).sum(-1)
        v_d = (W.reshape(cfg.d_in, H, C) * np.asarray(a_d, np.float32)[None]).sum(-1)
        return np.ascontiguousarray(np.concatenate([W, v_s, v_d], 1))

    wc = [fold(inputs["W1"], inputs["att_src1"], inputs["att_dst1"]),
          fold(inputs["W2"], inputs["att_src2"], inputs["att_dst2"])]

    xT = np.zeros((cfg.d_in, cfg.n_pad), np.float32)
    xT[:, :n] = x.T

    loops = np.arange(n, dtype=np.int64)
    src = np.concatenate([ei[0], loops]).astype(np.int64)
    dst = np.concatenate([ei[1], loops]).astype(np.int64)
    order = np.argsort(dst, kind="stable")
    src_s = src[order].astype(np.int32)
    dst_s = dst[order].astype(np.int32)
    dev = dst_s // cfg.npc
    rel = dst_s - dev * cfg.npc
    grp = rel >> 7
    hi_e = (src_s >= cfg.lo).astype(np.int64)
    # section index: (dev, grp, hi)
    sidx = (dev * cfg.groups + grp) * 2 + hi_e
    counts = np.bincount(sidx, minlength=cfg.n_cores * cfg.groups * 2)
    c2 = counts.reshape(-1, 2)
    ch_lo = max(1, int(math.ceil(c2[:, 0].max() / P)))
    ch_hi = max(1, int(math.ceil(c2[:, 1].max() / P)))
    ch = ch_lo + ch_hi
    # position within section (edges sorted by dst; stable order keeps
    # sections contiguous after argsort by sidx)
    order2 = np.argsort(sidx, kind="stable")
    src2 = src_s[order2]
    dst2 = dst_s[order2]
    sidx2 = sidx[order2]
    rel2 = (rel & 127)[order2]
    starts = np.zeros(len(counts), np.int64)
    starts[1:] = np.cumsum(counts)[:-1]
    pos_in_sec = np.arange(len(src2)) - starts[sidx2]
    dev2 = sidx2 // (cfg.groups * 2)
    grp2 = (sidx2 // 2) % cfg.groups
    hi2 = sidx2 & 1
    # slot position inside the group's padded [ch*P] edge list
    slot = np.where(hi2 == 0, pos_in_sec, ch_lo * P + pos_in_sec)
    gpos = grp2 * (ch * P) + slot     # position within device edge array

    GE = cfg.groups * ch * P
    src16 = np.zeros((cfg.n_cores, GE), np.int16)
    dst16 = np.zeros((cfg.n_cores, GE), np.int16)
    relf = np.full((cfg.n_cores, GE), -1.0, np.float32)
    parf = np.zeros((cfg.n_cores, GE), np.float32)
    src_adj = np.where(hi2 == 1, src2 - cfg.lo, src2).astype(np.int16)
    src16[dev2, gpos] = src_adj
    dst16[dev2, gpos] = (dst2 // 2).astype(np.int16)
    relf[dev2, gpos] = rel2.astype(np.float32)
    parf[dev2, gpos] = (dst2 & 1).astype(np.float32)

    l1b = np.asarray(inputs["lin1_b"], np.float32).reshape(4 * C, 1)
    l2b = np.broadcast_to(np.asarray(inputs["lin2_b"], np.float32), (P, C)).copy()
    b1 = np.broadcast_to(np.asarray(inputs["b1"], np.float32), (P, C)).copy()
    b2 = np.broadcast_to(np.asarray(inputs["b2"], np.float32), (P, C)).copy()

    in_maps = []
    for c in range(cfg.n_cores):
        xTs = np.zeros((cfg.d_in, cfg.npc_pad), np.float32)
        xTs[:, :cfg.npc] = x.T[:, c * cfg.npc:(c + 1) * cfg.npc]
        # wrapped int16 index arrays, per group
        esrc = np.concatenate(
            [_wrap16(src16[c, gi * ch * P:(gi + 1) * ch * P])
             for gi in range(cfg.groups)], axis=1)
        edst = np.concatenate(
            [_wrap16(dst16[c, gi * ch * P:(gi + 1) * ch * P])
             for gi in range(cfg.groups)], axis=1)
        # [P, g*ch] layouts: position i in group -> [i%128, g*ch + i//128]
        erel = np.concatenate(
            [relf[c, gi * ch * P:(gi + 1) * ch * P].reshape(ch, P).T
             for gi in range(cfg.groups)], axis=1)
        epar = np.concatenate(
            [parf[c, gi * ch * P:(gi + 1) * ch * P].reshape(ch, P).T
             for gi in range(cfg.groups)], axis=1)
        in_maps.append({
            "xT": xT, "xTs": xTs, "wc0": wc[0], "wc1": wc[1],
            "l1w": np.asarray(inputs["lin1_w"], np.float32),
            "l2w": np.asarray(inputs["lin2_w"], np.float32),
            "l1b": l1b, "l2b": l2b, "b0": b1, "b1": b2,
            "esrc": np.ascontiguousarray(esrc),
            "edst": np.ascontiguousarray(edst),
            "erel": np.ascontiguousarray(erel),
            "epar": np.ascontiguousarray(epar),
        })
    return in_maps, ch_lo, ch_hi


_CACHE = {}


def kernel(**inputs):
    cfg = Cfg()
    in_maps, ch_lo, ch_hi = preprocess(cfg, inputs)
    key = ("full", ch_lo, ch_hi)
    if key not in _CACHE:
        _CACHE[key] = build_program(cfg, ch_lo, ch_hi)
    nc = _CACHE[key]
    res = run_bass_kernel_spmd(nc, in_maps, list(range(cfg.n_cores))).results
    x_in = np.concatenate([res[c]["out_conv0"][:cfg.npc] for c in range(cfg.n_cores)])
    x_out = np.concatenate([res[c]["out_conv1"][:cfg.npc] for c in range(cfg.n_cores)])
    x_self = np.concatenate([res[c]["out_self"][:cfg.npc] for c in range(cfg.n_cores)])
    return (x_in, x_out, x_self)



# revision 13
# speedup vs baseline: 2.1028x; 2.1028x over previous
"""Distributed GAT (2x GATConv + MLP self-path) on 8 Trainium2 NeuronCores.

Strategy (dst-node graph parallelism, SPMD on 8 cores, v2):
  Host:
    - fold attention vectors into projection weights, cast x/weights to bf16,
      append self-loops, sort edges by dst, partition edges by 6272-node
      (128-aligned) dst blocks per core, group each core's dst nodes into
      128-node groups, order each group's edges [src<LO | src>=LO] with both
      sections padded to an even number of 128-edge chunks (global constants
      so the SPMD program is identical across cores); pad gather indices
      with -1 so the DMA skips them.
    - stream the per-chunk transposed one-hot dst-selector S_T (bf16) so the
      per-edge a_d term becomes a tiny on-device matmul instead of a gather.
  Device:
    - Phase S: MLP self path (bf16 matmuls) on own node block.
    - Phase A: a_d for own dst nodes from x_own @ v_d (kept in SBUF).
    - Phase T: projection table row(n) = [xp1 fp8|xp2 fp8|a_s1 bf16|a_s2
      bf16|pad] (1280 B) written as two tables (node < LO / >= LO so int16
      gather indices reach every row).
    - Phase E: per 128-dst-node group: one dma_gather per section for all
      edges (both convs share the row), scores exp(leakyrelu(a_s+a_d)-ln16)
      without max-subtraction (exact softmax identity, scale cancels),
      fp8 DoubleRow one-hot segment matmuls accumulate the weighted feature
      sum and the softmax denominator in one PSUM tile; normalisation,
      head-mean, bias, elu.
  Host: concatenate per-core dst blocks.
"""
import math
import numpy as np
import ml_dtypes

import concourse.bass as bass
import concourse.tile as tile
import concourse.mybir as mybir
import bass_rust
from concourse import bacc
from concourse.bass_utils import run_bass_kernel_spmd

AX_X = bass_rust.AxisListType.X
F32 = mybir.dt.float32
BF16 = mybir.dt.bfloat16
FP8 = mybir.dt.float8e4
U8 = mybir.dt.uint8
I16 = mybir.dt.int16
Act = mybir.ActivationFunctionType
Alu = mybir.AluOpType
DR = mybir.MatmulPerfMode.DoubleRow
P = 128
LN16 = float(np.log(16.0))


class Cfg:
    def __init__(self):
        self.n = 50000
        self.d_in = 256
        self.c = 16
        self.h = 32
        self.hd = self.h * self.c           # 512
        self.n_cores = 8
        self.npc = 6272                     # 128-aligned dst block per core
        self.groups = self.npc // P         # 49
        self.n_pad = self.npc * self.n_cores  # 50176
        self.nt = self.n_pad // P           # 392 node tiles
        self.kd = self.d_in // P            # 2
        self.lo = 32768
        self.ntl = self.lo // P             # 256 tiles in lo table
        self.n_hi = self.n_pad - self.lo    # 17408
        self.rowe = 1152                    # table row bf16 elements
        self.sub = 4                        # chunks per span (even)


def _elu(nc, pool, out_ap, in_ap, tag):
    shape = list(in_ap.shape)
    u = pool.tile(shape, F32, tag=tag + "_u")
    rl = pool.tile(shape, F32, tag=tag + "_r")
    nc.vector.tensor_scalar_min(out=u[:], in0=in_ap, scalar1=0.0)
    nc.scalar.activation(u[:], u[:], Act.Exp)
    nc.scalar.activation(rl[:], in_ap, Act.Relu)
    nc.vector.scalar_tensor_tensor(
        out=out_ap, in0=u[:], scalar=-1.0, in1=rl[:], op0=Alu.add, op1=Alu.add)


def build_program(cfg: Cfg, ch_lo: int, ch_hi: int, phases: str = "STE",
                  elevel: int = 4):
    nc = bacc.Bacc("TRN2", target_bir_lowering=False, debug=False,
                   num_devices=cfg.n_cores)
    G, H, C, HD = cfg.groups, cfg.h, cfg.c, cfg.hd
    ch = ch_lo + ch_hi
    ROWE = cfg.rowe
    SUB = cfg.sub

    t_xT = nc.dram_tensor("xT", [cfg.d_in, cfg.n_pad], BF16, kind="ExternalInput")
    t_xTs = nc.dram_tensor("xTs", [cfg.d_in, cfg.npc], BF16, kind="ExternalInput")
    t_wcat = nc.dram_tensor("wcat", [cfg.d_in, 1152], BF16, kind="ExternalInput")
    t_vd = nc.dram_tensor("vd", [cfg.d_in, 2 * H], BF16, kind="ExternalInput")
    t_l1w = nc.dram_tensor("l1w", [cfg.d_in, 4 * C], BF16, kind="ExternalInput")
    t_l2w = nc.dram_tensor("l2w", [4 * C, C], BF16, kind="ExternalInput")
    t_l1b = nc.dram_tensor("l1b", [4 * C, 1], F32, kind="ExternalInput")
    t_l2b = nc.dram_tensor("l2b", [P, C], F32, kind="ExternalInput")
    t_bcat = nc.dram_tensor("bcat", [P, 2 * C], F32, kind="ExternalInput")
    t_esrc = nc.dram_tensor("esrc", [P, G * ch * 8], I16, kind="ExternalInput")
    t_erel = nc.dram_tensor("erel", [P, G * ch], F32, kind="ExternalInput")
    t_sT = nc.dram_tensor("sT", [P, G * ch * P], BF16, kind="ExternalInput")

    t_oconv = [nc.dram_tensor(f"out_conv{i}", [cfg.npc, C], F32,
                              kind="ExternalOutput") for i in range(2)]
    t_oself = nc.dram_tensor("out_self", [cfg.npc, C], F32,
                             kind="ExternalOutput")

    t_tabL = nc.dram_tensor("tabL", [cfg.lo, ROWE], BF16)
    t_tabH = nc.dram_tensor("tabH", [cfg.n_hi, ROWE], BF16)

    with tile.TileContext(nc) as tc:
        import contextlib
        with contextlib.ExitStack() as ctx:
            cst = ctx.enter_context(tc.tile_pool(name="cst", bufs=1))
            ldp = ctx.enter_context(tc.tile_pool(name="ldp", bufs=2))
            xtp = ctx.enter_context(tc.tile_pool(name="xtp", bufs=2))
            tbp = ctx.enter_context(tc.tile_pool(name="tbp", bufs=3))
            wkp = ctx.enter_context(tc.tile_pool(name="wkp", bufs=2))

            # ---- constants ----
            iota_i = cst.tile([P, P], mybir.dt.int32, tag="ioi")
            nc.gpsimd.iota(iota_i[:], pattern=[[1, P]], base=0, channel_multiplier=0)
            iota_f = cst.tile([P, P], F32, tag="iof")
            nc.vector.tensor_copy(iota_f[:], iota_i[:])
            wcat = []
            for k in range(cfg.kd):
                w = cst.tile([P, 1152], BF16, tag=f"wc{k}")
                nc.sync.dma_start(out=w[:], in_=t_wcat[k * P:(k + 1) * P, :])
                wcat.append(w)
            vd = []
            for k in range(cfg.kd):
                w = cst.tile([P, 2 * H], BF16, tag=f"vd{k}")
                nc.sync.dma_start(out=w[:], in_=t_vd[k * P:(k + 1) * P, :])
                vd.append(w)
            l1w = []
            for k in range(cfg.kd):
                w = cst.tile([P, 4 * C], BF16, tag=f"l1w{k}")
                nc.sync.dma_start(out=w[:], in_=t_l1w[k * P:(k + 1) * P, :])
                l1w.append(w)
            l2w = cst.tile([4 * C, C], BF16, tag="l2w")
            nc.sync.dma_start(out=l2w[:], in_=t_l2w[:])
            l1b = cst.tile([4 * C, 1], F32, tag="l1b")
            nc.sync.dma_start(out=l1b[:], in_=t_l1b[:])
            l2b = cst.tile([P, C], F32, tag="l2b")
            nc.sync.dma_start(out=l2b[:], in_=t_l2b[:])
            bcat = cst.tile([P, 2, C], F32, tag="bcat")
            nc.sync.dma_start(out=bcat[:].rearrange("p u c -> p (u c)"),
                              in_=t_bcat[:])
            nl16 = cst.tile([P, 1], F32, tag="nl16")
            nc.gpsimd.memset(nl16[:], -LN16)
            erel = cst.tile([P, G * ch], F32, tag="erel")
            nc.sync.dma_start(out=erel[:], in_=t_erel[:])
            adn = cst.tile([P, G, 2 * H], BF16, tag="adn")

            # ---- Phase S: self path + Phase A: own-node a_d ----
            with tc.tile_pool(name="psS", bufs=2, space="PSUM") as psS, \
                 tc.tile_pool(name="psT", bufs=2, space="PSUM") as psT:
                blk = 0
                while "S" in phases and blk < cfg.npc:
                    bs = min(512, cfg.npc - blk)
                    x1p = psS.tile([4 * C, 512], F32, tag="x1p")
                    xk = []
                    for k in range(cfg.kd):
                        xts = xtp.tile([P, 512], BF16, tag="xts")
                        nc.sync.dma_start(out=xts[:, :bs],
                                          in_=t_xTs[k * P:(k + 1) * P, blk:blk + bs])
                        xk.append(xts)
                        nc.tensor.matmul(out=x1p[:, :bs], lhsT=l1w[k][:],
                                         rhs=xts[:, :bs],
                                         start=(k == 0), stop=(k == cfg.kd - 1))
                    x1s = wkp.tile([4 * C, 512], BF16, tag="x1s")
                    nc.vector.tensor_add(out=x1s[:, :bs], in0=x1p[:, :bs],
                                         in1=l1b[:].to_broadcast([4 * C, bs]))
                    _elu(nc, wkp, x1s[:, :bs], x1s[:, :bs], "se")
                    for m in range(bs // P):
                        gi = (blk + m * P) // P
                        o2p = psT.tile([P, C], F32, tag="o2p")
                        nc.tensor.matmul(out=o2p[:],
                                         lhsT=x1s[:, m * P:(m + 1) * P],
                                         rhs=l2w[:], start=True, stop=True)
                        o2s = wkp.tile([P, C], F32, tag="o2s")
                        nc.vector.tensor_add(out=o2s[:], in0=o2p[:], in1=l2b[:])
                        _elu(nc, wkp, o2s[:], o2s[:], "so")
                        nc.sync.dma_start(
                            out=t_oself[blk + m * P:blk + (m + 1) * P, :],
                            in_=o2s[:])
                        # own-node a_d for this 128-node group
                        adp = psT.tile([P, 2 * H], F32, tag="adp")
                        for k in range(cfg.kd):
                            nc.tensor.matmul(out=adp[:],
                                             lhsT=xk[k][:, m * P:(m + 1) * P],
                                             rhs=vd[k][:],
                                             start=(k == 0), stop=(k == cfg.kd - 1))
                        nc.vector.tensor_copy(out=adn[:, gi, :], in_=adp[:])
                    blk += bs

            # ---- Phase T: projection tables ----
            with tc.tile_pool(name="psA", bufs=2, space="PSUM") as psA:
                for ts4 in (range(0, cfg.nt, 4) if "T" in phases else []):
                    mt = min(4, cfg.nt - ts4)
                    xk = []
                    for k in range(cfg.kd):
                        xt = xtp.tile([P, 512], BF16, tag=f"xt{k}")
                        nc.sync.dma_start(
                            out=xt[:, :mt * P],
                            in_=t_xT[k * P:(k + 1) * P, ts4 * P:(ts4 + mt) * P])
                        xk.append(xt)
                    for m in range(mt):
                        ts = ts4 + m
                        pt = psA.tile([P, 1152], F32, tag="pt")
                        for k in range(cfg.kd):
                            for q0, q1 in ((0, 512), (512, 1024), (1024, 1152)):
                                nc.tensor.matmul(out=pt[:, q0:q1],
                                                 lhsT=xk[k][:, m * P:(m + 1) * P],
                                                 rhs=wcat[k][:, q0:q1],
                                                 start=(k == 0),
                                                 stop=(k == cfg.kd - 1))
                        # psum cols: [xp1 512 | a_s1 32 | a_d1 32 | xp2 512 |
                        #             a_s2 32 | a_d2 32]
                        stag = tbp.tile([P, ROWE], BF16, tag="stag")
                        nc.scalar.activation(
                            stag[:, 0:512], pt[:, 0:512], Act.Copy)
                        nc.scalar.activation(
                            stag[:, 512:1024], pt[:, 576:1088], Act.Copy)
                        nc.vector.tensor_copy(
                            out=stag[:, 1024:1056], in_=pt[:, 512:544])
                        nc.vector.tensor_copy(
                            out=stag[:, 1056:1088], in_=pt[:, 1088:1120])
                        if ts < cfg.ntl:
                            rows = t_tabL[ts * P:(ts + 1) * P, :]
                        else:
                            rows = t_tabH[(ts - cfg.ntl) * P:(ts - cfg.ntl + 1) * P, :]
                        nc.sync.dma_start(out=rows[:, 0:1088], in_=stag[:, 0:1088])

            # ---- Phase E: edge aggregation ----
            with tc.tile_pool(name="gat", bufs=2) as gat, \
                 tc.tile_pool(name="stp", bufs=2) as stp, \
                 tc.tile_pool(name="wsp", bufs=2) as wsp, \
                 tc.tile_pool(name="psE", bufs=2, space="PSUM") as psE, \
                 tc.tile_pool(name="psD", bufs=2, space="PSUM") as psD:
                npair = ch
                for gi in (range(G) if "E" in phases else []):
                    isrc = gat.tile([P, ch * 8], I16, tag="isrc")
                    nc.sync.dma_start(out=isrc[:],
                                      in_=t_esrc[:, gi * ch * 8:(gi + 1) * ch * 8])
                    sT = stp.tile([P, ch * P], BF16, tag="sT")
                    nc.sync.dma_start(out=sT[:],
                                      in_=t_sT[:, gi * ch * P:(gi + 1) * ch * P])
                    xpa = gat.tile([P, ch, ROWE], BF16, tag="xpa")
                    if gi == 0:
                        nc.vector.memset(xpa[:], 0.0)
                    for s0, s1, tab in ((0, ch_lo, t_tabL), (ch_lo, ch, t_tabH)):
                        for c0 in range(s0, s1, 8):
                            jw = min(8, s1 - c0)
                            nc.gpsimd.dma_gather(
                                out_ap=xpa[:, c0:c0 + jw, :], in_ap=tab[:],
                                idxs_ap=isrc[:, c0 * 8:(c0 + jw) * 8],
                                num_idxs=jw * P, num_idxs_reg=jw * P,
                                elem_size=ROWE)

                    out_ps = psE.tile([P, 1088], F32, tag="out_ps")
                    pair = 0
                    for c0 in range(0, ch, SUB):
                        jw = min(SUB, ch - c0)
                        if elevel < 2:
                            continue
                        # per-edge a_d via one-hot matmul from streamed S_T
                        ade = psD.tile([P, SUB, 2 * H], F32, tag="ade")
                        for j in range(jw):
                            nc.tensor.matmul(
                                out=ade[:, j, :],
                                lhsT=sT[:, (c0 + j) * P:(c0 + j + 1) * P],
                                rhs=adn[:, gi, :], start=True, stop=True)
                        # scores
                        scr = wsp.tile([P, SUB, 2 * H], F32, tag="scr")
                        nc.vector.tensor_add(
                            out=scr[:, :jw, :],
                            in0=xpa[:, c0:c0 + jw, 1024:1088],
                            in1=ade[:, :jw, :])
                        nc.vector.scalar_tensor_tensor(
                            out=scr[:, :jw, :], in0=scr[:, :jw, :], scalar=0.2,
                            in1=scr[:, :jw, :], op0=Alu.mult, op1=Alu.max)
                        wsc = wsp.tile([P, SUB, 1088], BF16, tag="wsc")
                        nc.scalar.activation(
                            wsc[:, :jw, 1024:1088], scr[:, :jw, :], Act.Exp,
                            bias=nl16[:], scale=1.0)
                        if elevel < 3:
                            continue
                        # weighted features
                        for i in range(2):
                            nc.vector.tensor_tensor(
                                out=wsc[:, :jw, i * HD:(i + 1) * HD].rearrange(
                                    "p s (h c) -> p s h c", h=H),
                                in0=xpa[:, c0:c0 + jw,
                                        i * HD:(i + 1) * HD].rearrange(
                                    "p s (h c) -> p s h c", h=H),
                                in1=wsc[:, :jw, 1024 + i * H:1024 + (i + 1) * H]
                                    .unsqueeze(3).to_broadcast([P, jw, H, C]),
                                op=Alu.mult)
                        # one-hot S (fp8) for this span
                        S = wsp.tile([P, SUB, P], BF16, tag="S")
                        nc.vector.tensor_tensor(
                            out=S[:, :jw, :],
                            in0=erel[:, gi * ch + c0:gi * ch + c0 + jw]
                                .unsqueeze(2).to_broadcast([P, jw, P]),
                            in1=iota_f[:].unsqueeze(1).to_broadcast([P, jw, P]),
                            op=Alu.is_equal)
                        for jj in range(jw):
                            for q0, q1 in ((0, 512), (512, 1024), (1024, 1088)):
                                nc.tensor.matmul(
                                    out=out_ps[:, q0:q1],
                                    lhsT=S[:, jj, :],
                                    rhs=wsc[:, jj, q0:q1],
                                    start=(pair == 0), stop=(pair == npair - 1))
                            pair += 1
                    if elevel < 4:
                        continue
                    # normalize + head mean + bias + elu
                    rsb = wkp.tile([P, 2 * H], F32, tag="rsb")
                    nc.scalar.activation(rsb[:], out_ps[:, 1024:1088],
                                         Act.Copy, scale=float(H), bias=1e-12)
                    nc.vector.reciprocal(rsb[:], rsb[:])
                    tmp = wkp.tile([P, 2, H, C], F32, tag="tmp")
                    nc.vector.tensor_tensor(
                        out=tmp[:],
                        in0=out_ps[:, 0:1024].rearrange(
                            "p (u h c) -> p u h c", u=2, h=H),
                        in1=rsb[:].rearrange("p (u h) -> p u h", u=2)
                            .unsqueeze(3).to_broadcast([P, 2, H, C]),
                        op=Alu.mult)
                    om = wkp.tile([P, 2, C], F32, tag="om")
                    nc.vector.reduce_sum(out=om[:],
                                         in_=tmp[:].rearrange("p u h c -> p u c h"),
                                         axis=AX_X)
                    nc.vector.tensor_add(out=om[:], in0=om[:], in1=bcat[:])
                    _elu(nc, wkp, om[:], om[:], "oe")
                    for i in range(2):
                        nc.sync.dma_start(
                            out=t_oconv[i][gi * P:(gi + 1) * P, :],
                            in_=om[:, i, :])
    nc.compile()
    return nc


def _wrap16(flat):
    """edge i -> [i%16, i//16], replicated to 128 partitions."""
    w = flat.reshape(-1, 16).T  # [16, len/16]
    return np.tile(w, (8, 1))


def preprocess(cfg: Cfg, inputs):
    n, H, C, G = cfg.n, cfg.h, cfg.c, cfg.groups
    x = np.asarray(inputs["x"], np.float32)
    ei = np.asarray(inputs["edge_index"])

    def fold(W, a_s, a_d):
        W = np.asarray(W, np.float32)
        v_s = (W.reshape(cfg.d_in, H, C) * np.asarray(a_s, np.float32)[None]).sum(-1)
        v_d = (W.reshape(cfg.d_in, H, C) * np.asarray(a_d, np.float32)[None]).sum(-1)
        return W, v_s, v_d

    W1, vs1, vd1 = fold(inputs["W1"], inputs["att_src1"], inputs["att_dst1"])
    W2, vs2, vd2 = fold(inputs["W2"], inputs["att_src2"], inputs["att_dst2"])
    # psum col layout: [xp1 512 | a_s1 32 | a_d1 32 | xp2 512 | a_s2 32 | a_d2 32]
    wcat = np.concatenate([W1, vs1, vd1, W2, vs2, vd2], 1).astype(
        ml_dtypes.bfloat16)
    vdcat = np.concatenate([vd1, vd2], 1).astype(ml_dtypes.bfloat16)

    xT = np.zeros((cfg.d_in, cfg.n_pad), ml_dtypes.bfloat16)
    xT[:, :n] = x.T.astype(ml_dtypes.bfloat16)

    loops = np.arange(n, dtype=np.int64)
    src = np.concatenate([ei[0], loops]).astype(np.int32)
    dst = np.concatenate([ei[1], loops]).astype(np.int32)
    order = np.argsort(dst, kind="stable")
    src_s = src[order]
    dst_s = dst[order]
    dev = dst_s // cfg.npc
    rel = dst_s - dev * cfg.npc
    grp = rel >> 7
    hi_e = (src_s >= cfg.lo).astype(np.int64)
    sidx = (dev * G + grp) * 2 + hi_e
    counts = np.bincount(sidx, minlength=cfg.n_cores * G * 2)
    c2 = counts.reshape(-1, 2)

    ch_lo = max(1, int(math.ceil(c2[:, 0].max() / P)))
    ch_hi = max(1, int(math.ceil(c2[:, 1].max() / P)))
    ch = ch_lo + ch_hi
    order2 = np.argsort(sidx, kind="stable")
    src2 = src_s[order2]
    sidx2 = sidx[order2]
    rel2 = (rel & 127)[order2]
    starts = np.zeros(len(counts), np.int64)
    starts[1:] = np.cumsum(counts)[:-1]
    pos_in_sec = np.arange(len(src2)) - starts[sidx2]
    dev2 = sidx2 // (G * 2)
    grp2 = (sidx2 // 2) % G
    hi2 = sidx2 & 1
    slot = np.where(hi2 == 0, pos_in_sec, ch_lo * P + pos_in_sec)
    gpos = grp2 * (ch * P) + slot

    GE = G * ch * P
    src16 = np.zeros((cfg.n_cores, GE), np.int16)
    relf = np.full((cfg.n_cores, GE), -1.0, np.float32)
    sTf = np.zeros((cfg.n_cores, P, GE), ml_dtypes.bfloat16)
    src_adj = np.where(hi2 == 1, src2 - cfg.lo, src2).astype(np.int16)
    src16[dev2, gpos] = src_adj
    relf[dev2, gpos] = rel2.astype(np.float32)
    sTf[dev2, rel2, gpos] = 1.0

    l1b = np.asarray(inputs["lin1_b"], np.float32).reshape(4 * C, 1)
    l2b = np.broadcast_to(np.asarray(inputs["lin2_b"], np.float32), (P, C)).copy()
    bcat = np.broadcast_to(
        np.concatenate([np.asarray(inputs["b1"], np.float32),
                        np.asarray(inputs["b2"], np.float32)]), (P, 2 * C)).copy()

    in_maps = []
    for c in range(cfg.n_cores):
        xTs = np.zeros((cfg.d_in, cfg.npc), ml_dtypes.bfloat16)
        lo_n = min(cfg.npc, max(0, n - c * cfg.npc))
        xTs[:, :lo_n] = xT[:, c * cfg.npc:c * cfg.npc + lo_n]
        esrc = np.concatenate(
            [_wrap16(src16[c, gi * ch * P:(gi + 1) * ch * P])
             for gi in range(G)], axis=1)
        erel = np.concatenate(
            [relf[c, gi * ch * P:(gi + 1) * ch * P].reshape(ch, P).T
             for gi in range(G)], axis=1)
        in_maps.append({
            "xT": xT, "xTs": xTs, "wcat": wcat, "vd": vdcat,
            "l1w": np.asarray(inputs["lin1_w"], np.float32).astype(
                ml_dtypes.bfloat16),
            "l2w": np.asarray(inputs["lin2_w"], np.float32).astype(
                ml_dtypes.bfloat16),
            "l1b": l1b, "l2b": l2b, "bcat": bcat,
            "esrc": np.ascontiguousarray(esrc),
            "erel": np.ascontiguousarray(erel),
            "sT": np.ascontiguousarray(sTf[c]),
        })
    return in_maps, ch_lo, ch_hi


_CACHE = {}


def kernel(**inputs):
    cfg = Cfg()
    in_maps, ch_lo, ch_hi = preprocess(cfg, inputs)
    key = ("v2", ch_lo, ch_hi)
    if key not in _CACHE:
        _CACHE[key] = build_program(cfg, ch_lo, ch_hi)
    nc = _CACHE[key]
    res = run_bass_kernel_spmd(nc, in_maps, list(range(cfg.n_cores))).results
    takes = [min(cfg.npc, cfg.n - c * cfg.npc) for c in range(cfg.n_cores)]
    x_in = np.concatenate([res[c]["out_conv0"][:takes[c]]
                           for c in range(cfg.n_cores)])
    x_out = np.concatenate([res[c]["out_conv1"][:takes[c]]
                            for c in range(cfg.n_cores)])
    x_self = np.concatenate([res[c]["out_self"][:takes[c]]
                             for c in range(cfg.n_cores)])
    return (x_in, x_out, x_self)
